# revision 3
# baseline (speedup 1.0000x reference)
"""BatchBlur: depthwise 15x15 conv with per-sample kernels, reflection pad 7.

x: (32, 3, 512, 512) f32, kernel: (32, 15, 15) f32 -> out (32, 3, 512, 512) f32.

Strategy: pure data parallel over batch, 4 samples (12 channel-images) per
core on 8 cores. Host: reflection-pad x to (., 526, 526), cast to fp16, and
build banded matrices A[s, i, j, m] = kern[s, i-m, j] (0 <= i-m < 15).

Device: the PE streams fp16 rhs at a fixed 512 B/cycle (256 elems/cycle),
so a matmul costs N*K_pad/256 cycles regardless of M or array tiling
(measured: K=128 -> 2 cols/cycle, K<=64 -> 4 cols/cycle, serial). The
optimal banded-conv mapping therefore maximizes M per stream on the full
128x128 array: M=103 output rows per strip (K = M+14 = 117 <= 128), 5
strips per image (4x103 + 100 = 512), one accumulating matmul per
horizontal tap j:
  out[m, n] += sum_i A[i, m] * xp[r0+i, n+j],  A[i, m] = kern[i-m, j]
 -> 12 images x 5 strips x 15 taps = 900 matmuls x 256 cycles ~= 96 us.
Single band: each strip's input rows are loaded once (contiguous DMA, no
shifted copies), column slices [j, j+512) of the 526-wide rows stay in
bounds. PSUM holds one f32 bank per strip; eviction casts to fp16 (DVE)
and stores fp16, halving store bytes; the host casts back to f32 (adds
~4e-4 relative error).
"""
import os
import sys

for _p in ("/opt/trn_rl_repo", "/root/.axon_site/_ro/trn_rl_repo"):
    if _p not in sys.path and os.path.isdir(_p):
        sys.path.insert(0, _p)

import numpy as np

import concourse.bass as bass
import concourse.mybir as mybir
import concourse.tile as tile
from concourse import bacc
from concourse.bass_utils import run_bass_kernel_spmd

L = 15           # blur kernel size
P = L // 2       # reflection pad
B, C, H, W = 32, 3, 512, 512
N_CORES = 8
BS = B // N_CORES            # samples per core (4)
NIMG = BS * C                # channel images per core (12)
HP, WP = H + 2 * P, W + 2 * P  # 526
M = 103                      # output rows per strip (K = M+14 = 117 <= 128)
R0S = [0, 103, 206, 309, 412]  # strip starts; last strip is 100 rows
MS = [M, M, M, M, H - 4 * M]   # rows per strip
N_WARMUP = 100               # dummy matmuls to release the HAM clock gate
PREFETCH = 3                 # input strips in flight ahead of compute

F16 = mybir.dt.float16
F32 = mybir.dt.float32

_program_cache = None


def _build_program():
    nc = bacc.Bacc("TRN2", target_bir_lowering=False, debug=False)
    xp_d = nc.dram_tensor("xp", [NIMG, HP, WP], F16, kind="ExternalInput").ap()
    a_d = nc.dram_tensor("a", [BS, 128, L, M], F16,
                         kind="ExternalInput").ap()
    out_d = nc.dram_tensor("out", [NIMG, H, W], F16,
                           kind="ExternalOutput").ap()

    units = [(img, s) for img in range(NIMG) for s in range(len(R0S))]

    with tile.TileContext(nc) as tc:
        with (
            tc.tile_pool(name="aconst", bufs=1) as apool,
            tc.tile_pool(name="warm", bufs=1) as wpool,
            tc.tile_pool(name="xin", bufs=6) as xpool,
            tc.tile_pool(name="oout", bufs=4) as opool,
            tc.tile_pool(name="psum", bufs=4, space="PSUM") as psum,
            tc.tile_pool(name="psumw", bufs=1, space="PSUM") as psumw,
        ):
            # HAM warm-up: a burst of matmuls on a zeroed scratch tile
            # releases the PE clock gate while the first input DMAs are in
            # flight.
            wsrc = wpool.tile([128, 64], mybir.dt.bfloat16)
            nc.gpsimd.memset(wsrc[:], 0.0)
            wacc = psumw.tile([64, 64], F32)
            for _ in range(N_WARMUP):
                nc.tensor.matmul(wacc[:], wsrc[:, :64], wsrc[:], start=True,
                                 stop=True)

            def load_unit(u, xt):
                img, s = units[u]
                kg = MS[s] + L - 1
                q = nc.sync if u % 2 == 0 else nc.gpsimd
                q.dma_start(out=xt[0:kg, :],
                            in_=xp_d[img, R0S[s]:R0S[s] + kg, :])

            xt = {}
            for u in range(PREFETCH):
                xt[u] = xpool.tile([128, WP], F16, tag="x", name="x")
                load_unit(u, xt[u])

            a_t = [
                apool.tile([128, L, M], F16, tag=f"a{s}", name=f"a{s}")
                for s in range(BS)
            ]
            for s in range(BS):
                nc.sync.dma_start(out=a_t[s][:], in_=a_d[s])

            for u, (img, s) in enumerate(units):
                if u + PREFETCH < len(units):
                    xt[u + PREFETCH] = xpool.tile([128, WP], F16, tag="x",
                                                  name="xn")
                    load_unit(u + PREFETCH, xt[u + PREFETCH])
                ms = MS[s]
                kg = ms + L - 1
                smp = img // C
                acc = psum.tile([128, W], F32, tag="ps", name="ps")
                for j in range(L):
                    nc.tensor.matmul(
                        acc[0:ms, :],
                        a_t[smp][0:kg, j, 0:ms],
                        xt[u][0:kg, j:j + W],
                        start=(j == 0),
                        stop=(j == L - 1),
                    )
                o_t = opool.tile([128, W], F16, tag="o", name="o")
                nc.vector.tensor_copy(out=o_t[0:ms, :], in_=acc[0:ms, :])
                nc.scalar.dma_start(out=out_d[img, R0S[s]:R0S[s] + ms, :],
                                    in_=o_t[0:ms, :])
                del xt[u]
    nc.compile()
    return nc


def prepare_in_maps(x: np.ndarray, kern: np.ndarray) -> list:
    # host-side reflection pad, cast to fp16 for half the DMA bytes
    xp = np.pad(x, ((0, 0), (0, 0), (P, P), (P, P)), mode="reflect")
    xp = np.ascontiguousarray(
        xp.reshape(B * C, HP, WP).astype(np.float16))

    # banded matrices A[s, i, j, m] = kern[s, i-m, j]
    kern16 = kern.astype(np.float16)
    a_all = np.zeros((B, 128, L, M), dtype=np.float16)
    m_idx = np.arange(M)
    for dy in range(L):
        a_all[:, m_idx + dy, :, m_idx] = kern16[:, dy, :]

    return [
        {
            "xp": xp[c * NIMG:(c + 1) * NIMG],
            "a": a_all[c * BS:(c + 1) * BS],
        }
        for c in range(N_CORES)
    ]


def kernel(x: np.ndarray, kernel: np.ndarray) -> np.ndarray:
    global _program_cache
    x = np.asarray(x, dtype=np.float32)
    kern = np.asarray(kernel, dtype=np.float32)

    in_maps = prepare_in_maps(x, kern)
    if _program_cache is None:
        _program_cache = _build_program()
    nc = _program_cache

    res = run_bass_kernel_spmd(nc, in_maps, core_ids=list(range(N_CORES)))
    out = np.concatenate([r["out"] for r in res.results], axis=0)
    return out.reshape(B, C, H, W).astype(np.float32)
